# revision 20
# baseline (speedup 1.0000x reference)
"""Block-local attention + FFN Trainium2 kernel (8 NeuronCores, SPMD).

v2 design notes (vs the f32r baseline at 360us):
- Everything the PE touches is bf16: enables FWL (fast weight load), avoids
  the FP32-HIGH hang-guard that disables it, and halves LDWEIGHTS time.
- The PE HAM clock gate needs ~3.4us of continuous busy to reach 2.4 GHz;
  the schedule keeps an unbroken matmul stream (attention of block t-1 and
  FFN of block t-2 interleaved at head granularity between the LN chains).
- The axial softmax bias is rank-20 (4+8+8 one-hot factors): it rides for
  free inside the scores matmul as extra contraction rows (k/q tiles are
  augmented with the V_h / U factors in otherwise-zero partitions).
- LN stats are computed with an all-ones [128,128] stationary, so the
  per-token sums arrive in PSUM already replicated across all partitions:
  no broadcast matmul is needed to apply LN, and every matmul in the kernel
  has tile_size (128,128) (no PE tiling-mode switches).
- rstd = exp(-0.5*ln(var+eps)) so the only ACT table set used is
  natural_log_exp_and_others (ln/exp/relu/copy/identity in one set; the
  baseline's Sqrt/Rsqrt forced ~2.7us table swaps).
- softmax denominators: ones-column on V, gathered by DMA, inverted with
  reciprocal_approx_fast, broadcast across partitions with a one-hot
  selector matmul, applied per-head by DVE reading PSUM directly.
"""

import numpy as np

import concourse.bass as bass
import concourse.mybir as mybir
import concourse.tile as tile

F32 = mybir.dt.float32
F32R = mybir.dt.float32r
BF16 = mybir.dt.bfloat16
AF = mybir.ActivationFunctionType
ALU = mybir.AluOpType

# Problem constants (hardcoded per the harness contract).
B, C, T, H, W = 2, 512, 8, 32, 32
BT, BH, BW = 4, 8, 8                 # block dims (t, h, w)
NH, DA = 8, 64
EPS = 1e-5
ST, SH, SW = T // BT, H // BH, W // BW
THW = BT * BH * BW                   # 256 tokens per block
NB = B * ST * SH * SW                # 64 blocks
NCORES = 8
NBLK = NB // NCORES                  # 8 blocks per core
KC = C // 128                        # 4 channel chunks
TOK = THW                            # 256
KB = 20                              # rank of the axial bias (4+8+8)


def _r(ap):
    return ap.bitcast(F32R)


def _legalize_waits(nc, limit=1):
    """This container's walrus rejects instructions carrying more than ~2
    sem-wait commands (setupSyncWait: "Too many sync wait commands"). Hoist
    excess waits onto preceding single-wait NOPs on the same engine."""
    for f in nc.m.functions:
        for blk in f.blocks:
            newl = []
            changed = False
            for ins in blk.instructions:
                si = ins.sync_info
                waits = list(si.on_wait) if (si is not None and si.on_wait) else []
                if len(waits) > limit:
                    changed = True
                    for k in range(0, len(waits), limit):
                        nop = mybir.InstNoOp(
                            name=f"{ins.name}-ws{k}",
                            sync_info=mybir.SyncInfo(
                                on_wait=list(waits[k:k + limit]), on_update=[]),
                            bass_nofuse=True,
                            engine=ins.engine,
                        )
                        try:
                            nc.register_instruction(nop, overwrite=True)
                        except Exception:
                            pass
                        newl.append(nop)
                    si.on_wait = []
                newl.append(ins)
            if changed:
                try:
                    blk.instructions = newl
                except Exception:
                    blk.instructions.clear()
                    for i in newl:
                        blk.instructions.append(i)


def build_kernel(bq_nz, bk_nz, bv_nz, b1_nz, b2_nz):
    nc = bass.Bass()

    xs_d = nc.declare_dram_parameter("xs", [NBLK, 128, KC, TOK], BF16, isOutput=False)
    wq_d = nc.declare_dram_parameter("wq", [KC, 128, 512], BF16, isOutput=False)
    wk_d = nc.declare_dram_parameter("wk", [KC, 128, 512], BF16, isOutput=False)
    wv_d = nc.declare_dram_parameter("wv", [KC, 128, 512], BF16, isOutput=False)
    wp_d = nc.declare_dram_parameter("wp", [KC, 128, 512], BF16, isOutput=False)
    w1_d = nc.declare_dram_parameter("w1", [KC, 128, 512], BF16, isOutput=False)
    w2_d = nc.declare_dram_parameter("w2", [KC, 128, 512], BF16, isOutput=False)
    uc_d = nc.declare_dram_parameter("uconst", [KB, TOK], BF16, isOutput=False)
    vc_d = nc.declare_dram_parameter("vconst", [NH, KB, TOK], BF16, isOutput=False)
    sel_d = nc.declare_dram_parameter("sel", [128, KC, 128], BF16, isOutput=False)
    br_d = nc.declare_dram_parameter("brows", [128, 16], F32, isOutput=False)
    bv_d = nc.declare_dram_parameter("bvrow", [1, 512], BF16, isOutput=False)
    out_d = nc.declare_dram_parameter("out", [NBLK, 128, KC, TOK], F32, isOutput=True)

    from contextlib import ExitStack

    with nc.allow_low_precision(reason="bf16 matmul/activation pipeline"), \
            tile.TileContext(nc) as tc, ExitStack() as ctx:
        cp = ctx.enter_context(tc.tile_pool(name="const", bufs=1))
        xp = ctx.enter_context(tc.tile_pool(name="xp", bufs=3))
        sp = ctx.enter_context(tc.tile_pool(name="sp", bufs=2))
        lnp = ctx.enter_context(tc.tile_pool(name="lnp", bufs=2))
        etp = ctx.enter_context(tc.tile_pool(name="etp", bufs=4))
        op_ = ctx.enter_context(tc.tile_pool(name="op", bufs=3))
        outp = ctx.enter_context(tc.tile_pool(name="outp", bufs=2))
        d8p = ctx.enter_context(tc.tile_pool(name="d8p", bufs=2))
        psA = ctx.enter_context(tc.tile_pool(name="psA", bufs=3, space="PSUM"))
        psS = ctx.enter_context(tc.tile_pool(name="psS", bufs=2, space="PSUM"))
        psV = ctx.enter_context(tc.tile_pool(name="psV", bufs=1, space="PSUM"))
        psR = ctx.enter_context(tc.tile_pool(name="psR", bufs=1, space="PSUM"))

        # --- persistent constants ---
        wq_s = cp.tile([128, KC, 512], BF16)
        wk_s = cp.tile([128, KC, 512], BF16)
        wv_s = cp.tile([128, KC, 512], BF16)
        wp_s = cp.tile([128, KC, 512], BF16)
        w1_s = cp.tile([128, KC, 512], BF16)
        w2_s = cp.tile([128, KC, 512], BF16)
        for w_s, w_d in ((wq_s, wq_d), (wk_s, wk_d), (wv_s, wv_d),
                         (wp_s, wp_d), (w1_s, w1_d), (w2_s, w2_d)):
            for kc in range(KC):
                nc.gpsimd.dma_start(w_s[:, kc, :], w_d[kc])
        sel_s = cp.tile([128, KC, 128], BF16)
        nc.gpsimd.dma_start(sel_s[:], sel_d[:])
        br_s = cp.tile([128, 16], F32)
        nc.gpsimd.dma_start(br_s[:], br_d[:])
        bvr_s = cp.tile([1, 512], BF16)
        nc.gpsimd.dma_start(bvr_s[0:1, :], bv_d[:])

        allones = cp.tile([128, 128], BF16)
        nc.vector.memset(allones[:], 1.0)
        ones_row = cp.tile([1, 128], BF16)
        nc.vector.memset(ones_row[0:1, :], 1.0)
        eps_col = cp.tile([128, 1], F32)
        nc.vector.memset(eps_col[:], EPS)
        first_x = [None]

        # Augmented q/k tiles (double buffered by block parity). Chunk h:
        #  even h: rows 0-63 q/k of head h, rows 64-83 U / V_h, rest zero
        #  odd  h: rows 64-127 q/k of head h, rows 0-19 U / V_h, rest zero
        qaug = cp.tile([128, 2, NH, TOK], BF16)
        kaug = cp.tile([128, 2, NH, TOK], BF16)
        v65 = cp.tile([128, 2, 2, NH, 65], BF16)
        nc.vector.memset(qaug[:], 0.0)
        nc.vector.memset(kaug[:], 0.0)
        nc.gpsimd.memset(v65[:], 0.0)
        nc.gpsimd.memset(v65[:, :, :, :, 64:65], 1.0)
        # scores^T[key m, query n] needs +bias[n, m] = sum_a V[h,n,a]*U[m,a]:
        # the stationary (kaug) carries U over key tokens, the moving (qaug)
        # carries V_h over query tokens.
        for b in range(2):
            for h in range(NH):
                po = 64 if h % 2 == 0 else 0
                nc.scalar.dma_start(qaug[po:po + KB, b, h, :], vc_d[h])
                nc.scalar.dma_start(kaug[po:po + KB, b, h, :], uc_d[:])
        # Persistent selector-matmul moving tiles (parity double-buffered):
        # unused rows must stay zero (they multiply zero weights, but NaN*0
        # would poison the psum), so these cannot rotate through a pool.
        d8r_s = []
        for g in range(2):
            d8rt = cp.tile([128, 2, TOK], BF16, name=f"d8rt{g}")
            nc.vector.memset(d8rt[:], 0.0)
            d8r_s.append(d8rt)

        def _ln_chain(ps_st, tag):
            """Replicated per-token LN coefficients: rb = rstd, mrb = mean*rstd,
            both [128, TOK] (identical rows)."""
            m = lnp.tile([128, TOK], BF16, tag=f"m{tag}")
            nc.scalar.activation(m[:], ps_st[:, 0:256], AF.Copy, scale=1.0 / C)
            m2 = lnp.tile([128, TOK], F32, tag=f"m2{tag}")
            nc.gpsimd.tensor_mul(m2[:], m[:], m[:])
            var = lnp.tile([128, TOK], F32, tag=f"var{tag}")
            nc.vector.scalar_tensor_tensor(var[:], ps_st[:, 256:512], 1.0 / C,
                                           m2[:], op0=ALU.mult, op1=ALU.subtract)
            lv = lnp.tile([128, TOK], F32, tag=f"lv{tag}")
            nc.scalar.activation(lv[:], var[:], AF.Ln, bias=eps_col[:, 0:1])
            rb = lnp.tile([128, TOK], BF16, tag=f"rb{tag}")
            nc.scalar.activation(rb[:], lv[:], AF.Exp, scale=-0.5)
            mrb = lnp.tile([128, TOK], BF16, tag=f"mrb{tag}")
            nc.gpsimd.tensor_mul(mrb[:], m[:], rb[:])
            return {f"rb{tag}": rb, f"mrb{tag}": mrb}

        def _ln_apply(src, rb, mrb, tag):
            zh = sp.tile([128, KC, TOK], BF16, tag=f"zh{tag}")
            for kc in range(KC):
                eng = nc.vector if kc < 2 else nc.gpsimd
                eng.tensor_mul(zh[:, kc, :], src[:, kc, :], rb[:])
                eng.tensor_sub(zh[:, kc, :], zh[:, kc, :], mrb[:])
            return zh

        def s0_load(t):
            """Prefetch x for block t (issued one iteration ahead)."""
            st = {"b": t}
            x = xp.tile([128, KC, TOK], BF16, tag="x")
            nc.sync.dma_start(x[:], xs_d[t])
            st["x"] = x
            return st

        def s0_stats(st):
            """LN1 stats matmuls + LN1 scalar chain."""
            x = st["x"]
            sq = sp.tile([128, KC, TOK], BF16, tag="sq1")
            nc.gpsimd.tensor_mul(sq[:], x[:], x[:])
            ps_st = psA.tile([128, 512], F32, tag="mm")
            for kc in range(KC):
                nc.tensor.matmul(ps_st[:, 0:256], allones[:], x[:, kc, :],
                                 start=(kc == 0), stop=(kc == KC - 1))
            for kc in range(KC):
                nc.tensor.matmul(ps_st[:, 256:512], allones[:], sq[:, kc, :],
                                 start=(kc == 0), stop=(kc == KC - 1))
            st.update(_ln_chain(ps_st, "1"))

        def s1_qkv(st, prev):
            t = st["b"]
            zh = _ln_apply(st["x"], st["rb1"], st["mrb1"], "1")
            # q, k GEMMs -> augmented per-head tiles; the selector matmuls of
            # the previous block ride between GEMMs so their ACT chain is
            # covered by matmul work.
            for gsel, (aug, w_s, bcol, nz) in enumerate((
                    (qaug, wq_s, 0, bq_nz), (kaug, wk_s, 4, bk_nz))):
                for pair in range(2):
                    ps = psA.tile([128, 512], F32, tag="mm")
                    for half in range(2):
                        mf = pair * 2 + half
                        o = ps[:, half * 256:(half + 1) * 256]
                        for kc in range(KC):
                            nc.tensor.matmul(
                                o, w_s[:, kc, mf * 128:(mf + 1) * 128],
                                zh[:, kc, :],
                                start=(kc == 0), stop=(kc == KC - 1))
                    # copies into aug chunks (heads 4*pair .. 4*pair+3)
                    c0 = 4 * pair
                    if nz:
                        for half in range(2):
                            mf = pair * 2 + half
                            col = bcol + mf
                            nc.vector.tensor_scalar_add(
                                aug[0:64, t % 2, c0 + 2 * half, :],
                                ps[0:64, half * 256:(half + 1) * 256],
                                br_s[0:64, col:col + 1])
                            nc.vector.tensor_scalar_add(
                                aug[64:128, t % 2, c0 + 2 * half + 1, :],
                                ps[64:128, half * 256:(half + 1) * 256],
                                br_s[64:128, col:col + 1])
                    else:
                        src = ps[:].rearrange("p (a b) -> p a b", a=2)
                        nc.vector.tensor_copy(
                            aug[0:64, t % 2, c0:c0 + 3:2, :], src[0:64])
                        nc.vector.tensor_copy(
                            aug[64:128, t % 2, c0 + 1:c0 + 4:2, :], src[64:128])
                if prev is not None and pair == 1:
                    s3_rb(prev, gsel)
            # v GEMM -> v65 (tokens on partitions)
            for tcx in range(2):
                ps = psA.tile([128, 512], F32, tag="mm")
                for kc in range(KC):
                    nc.tensor.matmul(
                        ps[:], zh[:, kc, tcx * 128:(tcx + 1) * 128],
                        wv_s[:, kc, :],
                        start=(kc == 0), stop=(kc == KC - 1 and not bv_nz))
                if bv_nz:
                    nc.tensor.matmul(ps[:], ones_row[0:1, :], bvr_s[0:1, :],
                                     start=False, stop=True)
                nc.scalar.activation(
                    v65[:, t % 2, tcx, :, 0:64],
                    ps[:].rearrange("p (h e) -> p h e", h=NH), AF.Copy)

        def s2_scores(st, h):
            t = st["b"]
            ps_s = psS.tile([128, 512], F32, tag="ss")
            for kt in range(2):
                nc.tensor.matmul(
                    ps_s[:, kt * 256:(kt + 1) * 256],
                    kaug[:, t % 2, h, kt * 128:(kt + 1) * 128],
                    qaug[:, t % 2, h, :], start=True, stop=True)
            e = etp.tile([128, 2, TOK], BF16, tag="e")
            nc.scalar.activation(e[:], ps_s[:].rearrange("p (a b) -> p a b", a=2),
                                 AF.Exp)
            st[f"e{h}"] = e

        def s2_av(st, h):
            """AV matmul for one head; on the last head of each 4-head group,
            gather the denominators by DMA and free the psum via a copy."""
            t = st["b"]
            g = h // 4
            if h % 4 == 0:
                st[f"pav{g}"] = psV.tile([65, 4, TOK], F32, tag="av", name=f"pav{g}")
            pav = st[f"pav{g}"]
            e = st.pop(f"e{h}")
            for kt in range(2):
                nc.tensor.matmul(pav[:, h % 4, :], v65[:, t % 2, kt, h, :],
                                 e[:, kt, :], start=(kt == 0), stop=(kt == 1))
            if h % 4 == 3:
                # copy the group's AV results (and denominator row 64) off
                # PSUM so the single psV buffer can be reused by the next
                # group without waiting on the normalization chain.
                atu = sp.tile([65, 4, TOK], BF16, tag=f"atu{g}", name="atu")
                if g == 0:
                    nc.vector.tensor_copy(atu[:], pav[:])
                else:
                    nc.scalar.activation(atu[:], pav[:], AF.Copy)
                st[f"atu{g}"] = atu
                d8 = d8p.tile([64, TOK], BF16, tag=f"d8{g}")
                nc.sync.dma_start(d8[32 * g:32 * g + 4, :], atu[64:65, :, :])
                st[f"d8{g}"] = d8

        def s2_recip(st, g):
            # 1/d = exp(-ln(d)) keeps the whole kernel on one ACT table set
            # and avoids the fixed-cost (1.7us) DVE reciprocal.
            lnd = d8p.tile([64, TOK], F32, tag=f"d8f{g}")
            nc.scalar.activation(lnd[32 * g:32 * g + 4, :],
                                 st[f"d8{g}"][32 * g:32 * g + 4, :], AF.Ln)
            d8r = d8r_s[g][:, st["b"] % 2, :]
            nc.scalar.activation(d8r[32 * g:32 * g + 4, :],
                                 lnd[32 * g:32 * g + 4, :], AF.Exp, scale=-1.0)
            st[f"d8r{g}"] = d8r

        def s3_rb(st, g):
            """Selector matmuls broadcast 1/d across partitions for group g,
            then normalize that group's heads into aT."""
            if g == 0:
                st["aT"] = sp.tile([128, KC, TOK], BF16, tag="aT", name="aT")
            aT = st["aT"]
            ps_rb = psR.tile([128, 512], F32, tag="rb")
            for ci in range(2):
                mf = 2 * g + ci
                nc.tensor.matmul(ps_rb[:, ci * 256:(ci + 1) * 256],
                                 sel_s[:, mf, :], st[f"d8r{g}"],
                                 start=True, stop=True)
            atu = st[f"atu{g}"]
            eng = nc.vector
            for hh in range(4):
                h = 4 * g + hh
                mf, po = h // 2, (h % 2) * 64
                ci = mf - 2 * g
                eng.tensor_mul(aT[po:po + 64, mf, :], atu[0:64, hh, :],
                               ps_rb[po:po + 64, ci * 256:(ci + 1) * 256])

        def s3_norm_proj(st):
            aT = st["aT"]
            o = op_.tile([128, KC, TOK], BF16, tag="o")
            for pair in range(2):
                ps_o = psA.tile([128, 512], F32, tag="mm")
                for half in range(2):
                    mc = pair * 2 + half
                    dst = ps_o[:, half * 256:(half + 1) * 256]
                    for fc in range(KC):
                        nc.tensor.matmul(
                            dst, wp_s[:, fc, mc * 128:(mc + 1) * 128],
                            aT[:, fc, :],
                            start=(fc == 0), stop=(fc == KC - 1))
                nc.vector.tensor_add(
                    o[:, pair * 2:(pair + 1) * 2, :],
                    ps_o[:].rearrange("p (a b) -> p a b", a=2),
                    st["x"][:, pair * 2:(pair + 1) * 2, :])
            st["o"] = o
            # LN2 stats
            sq = sp.tile([128, KC, TOK], BF16, tag="sq2")
            nc.gpsimd.tensor_mul(sq[:], o[:], o[:])
            ps_st = psA.tile([128, 512], F32, tag="mm")
            for kc in range(KC):
                nc.tensor.matmul(ps_st[:, 0:256], allones[:], o[:, kc, :],
                                 start=(kc == 0), stop=(kc == KC - 1))
            for kc in range(KC):
                nc.tensor.matmul(ps_st[:, 256:512], allones[:], sq[:, kc, :],
                                 start=(kc == 0), stop=(kc == KC - 1))
            st.update(_ln_chain(ps_st, "2"))

        def s4_apply(st):
            st["yh"] = _ln_apply(st["o"], st["rb2"], st["mrb2"], "2")

        def s4_ffn_a(st, pair):
            if pair == 0:
                st["h1"] = sp.tile([128, KC, TOK], BF16, tag="h1", name="h1")
            yh, h1 = st["yh"], st["h1"]
            ps_h = psA.tile([128, 512], F32, tag="mm")
            for half in range(2):
                mf = pair * 2 + half
                o = ps_h[:, half * 256:(half + 1) * 256]
                for kc in range(KC):
                    nc.tensor.matmul(
                        o, w1_s[:, kc, mf * 128:(mf + 1) * 128],
                        yh[:, kc, :],
                        start=(kc == 0), stop=(kc == KC - 1))
            if b1_nz:
                for half in range(2):
                    mf = pair * 2 + half
                    nc.scalar.activation(
                        h1[:, mf, :], ps_h[:, half * 256:(half + 1) * 256],
                        AF.Relu, bias=br_s[:, 8 + mf:8 + mf + 1])
            else:
                nc.scalar.activation(
                    h1[:, pair * 2:(pair + 1) * 2, :],
                    ps_h[:].rearrange("p (a b) -> p a b", a=2), AF.Relu)

        def s4_ffn_b(st, pair):
            h1, o_sb = st["h1"], st["o"]
            if pair == 0:
                st["out"] = outp.tile([128, KC, TOK], F32, tag="out", name="outsb")
            out_sb = st["out"]
            ps_y = psA.tile([128, 512], F32, tag="mm")
            for half in range(2):
                mc = pair * 2 + half
                o = ps_y[:, half * 256:(half + 1) * 256]
                for fc in range(KC):
                    nc.tensor.matmul(
                        o, w2_s[:, fc, mc * 128:(mc + 1) * 128],
                        h1[:, fc, :],
                        start=(fc == 0), stop=(fc == KC - 1))
            if b2_nz:
                for half in range(2):
                    mc = pair * 2 + half
                    nc.vector.scalar_tensor_tensor(
                        out_sb[:, mc, :],
                        ps_y[:, half * 256:(half + 1) * 256],
                        br_s[:, 12 + mc:12 + mc + 1],
                        o_sb[:, mc, :], op0=ALU.add, op1=ALU.add)
            else:
                nc.vector.tensor_add(
                    out_sb[:, pair * 2:(pair + 1) * 2, :],
                    ps_y[:].rearrange("p (a b) -> p a b", a=2),
                    o_sb[:, pair * 2:(pair + 1) * 2, :])
            if pair == 1:
                nc.sync.dma_start(out_d[st["b"] % NBLK], out_sb[:])

        def attn_block(prev, old):
            """Attention of block prev interleaved (at head granularity) with
            the FFN of block old; keeps the PE stream gap-free while exp and
            the denominator chain run on ACT/DVE/DMA."""
            if prev is None:
                if old is not None:
                    for p in range(2):
                        s4_ffn_a(old, p)
                    for p in range(2):
                        s4_ffn_b(old, p)
                return
            for g in range(2):
                h0 = 4 * g
                s2_scores(prev, h0)
                s2_scores(prev, h0 + 1)
                s2_scores(prev, h0 + 2)
                s2_av(prev, h0)
                s2_scores(prev, h0 + 3)
                s2_av(prev, h0 + 1)
                if old is not None:
                    (s4_ffn_a if g == 0 else s4_ffn_b)(old, 0)
                s2_av(prev, h0 + 2)
                s2_av(prev, h0 + 3)
                s2_recip(prev, g)
                if old is not None:
                    (s4_ffn_a if g == 0 else s4_ffn_b)(old, 1)

        # --- software pipeline across blocks ---
        blocks = {0: s0_load(0)}
        for t in range(NBLK):
            s0_stats(blocks[t])
            if t - 2 in blocks:
                s4_apply(blocks[t - 2])
            if t + 1 < NBLK:
                blocks[t + 1] = s0_load(t + 1)
            attn_block(blocks.get(t - 1), blocks.get(t - 2))
            if t - 2 in blocks:
                blocks.pop(t - 2)
            s1_qkv(blocks[t], blocks.get(t - 1))
            if t - 1 in blocks:
                s3_norm_proj(blocks[t - 1])
        old = blocks.pop(NBLK - 2)
        last = blocks.pop(NBLK - 1)
        s4_apply(old)
        attn_block(last, old)
        for g in range(2):
            s3_rb(last, g)
        s3_norm_proj(last)
        s4_apply(last)
        for p in range(2):
            s4_ffn_a(last, p)
        for p in range(2):
            s4_ffn_b(last, p)

    _legalize_waits(nc)
    return nc


_CACHE = {}


def _get_nc(flags):
    if flags not in _CACHE:
        _CACHE[flags] = build_kernel(*flags)
    return _CACHE[flags]


def _bias_factors(dt_bank, dh_bank, dw_bank):
    """U [THW, KB] one-hot factor; V [NH, THW, KB] such that
    bias_h[i, j] = sum_a V[h, i, a] * U[j, a]."""
    ar = np.arange(THW)
    tt = ar // (BH * BW)
    hh = (ar // BW) % BH
    ww = ar % BW
    U = np.zeros((THW, KB), np.float32)
    U[ar, tt] = 1.0
    U[ar, 4 + hh] = 1.0
    U[ar, 12 + ww] = 1.0
    V = np.zeros((NH, THW, KB), np.float32)
    for a in range(BT):
        V[:, :, a] = dt_bank[:, (tt - a) + (BT - 1)]
    for b_ in range(BH):
        V[:, :, 4 + b_] = dh_bank[:, (hh - b_) + (BH - 1)]
    for c_ in range(BW):
        V[:, :, 12 + c_] = dw_bank[:, (ww - c_) + (BW - 1)]
    return U, V


def prepare(x, dt_bank, dh_bank, dw_bank, ln1_g, ln1_b, w_q, w_k, w_v,
            w_proj, ln2_g, ln2_b, w1, b1, w2, b2):
    """Host-side prep: returns (flags, in_maps)."""
    import ml_dtypes
    bf16 = ml_dtypes.bfloat16
    f = np.float32
    x = np.asarray(x, f)

    # block split: (B,C,T,H,W) -> (NB, C, THW), channels-major
    xb = x.reshape(B, C, ST, BT, SH, BH, SW, BW)
    xb = xb.transpose(0, 2, 4, 6, 1, 3, 5, 7).reshape(NB, C, THW)
    xb = np.ascontiguousarray(xb).reshape(NB, KC, 128, TOK)
    xb = np.ascontiguousarray(xb.transpose(0, 2, 1, 3)).astype(bf16)

    scale = 1.0 / np.sqrt(DA)
    wqf = np.asarray(w_q, f).transpose(1, 0, 2).reshape(C, NH * DA)
    wkf = np.asarray(w_k, f).transpose(1, 0, 2).reshape(C, NH * DA)
    wvf = np.asarray(w_v, f).transpose(1, 0, 2).reshape(C, NH * DA)
    g1 = np.asarray(ln1_g, f)[:, None]
    b1v = np.asarray(ln1_b, f)
    wq_e = ((g1 * wqf) * scale).reshape(KC, 128, 512).astype(bf16)
    wk_e = (g1 * wkf).reshape(KC, 128, 512).astype(bf16)
    wv_e = (g1 * wvf).reshape(KC, 128, 512).astype(bf16)
    bq = (b1v @ wqf) * scale
    bk = b1v @ wkf
    bv = b1v @ wvf
    wp_e = np.ascontiguousarray(np.asarray(w_proj, f).T).reshape(
        KC, 128, 512).astype(bf16)
    g2 = np.asarray(ln2_g, f)[:, None]
    b2v = np.asarray(ln2_b, f)
    w1t = np.asarray(w1, f).T
    w1_e = (g2 * w1t).reshape(KC, 128, 512).astype(bf16)
    b1p = b2v @ w1t + np.asarray(b1, f)
    w2_e = np.ascontiguousarray(np.asarray(w2, f).T).reshape(
        KC, 128, 512).astype(bf16)
    b2p = np.asarray(b2, f)

    brows = np.zeros((128, 16), f)
    brows[:, 0:4] = bq.reshape(KC, 128).T
    brows[:, 4:8] = bk.reshape(KC, 128).T
    brows[:, 8:12] = b1p.reshape(KC, 128).T
    brows[:, 12:16] = b2p.reshape(KC, 128).T
    bvrow = np.ascontiguousarray(bv.reshape(1, 512)).astype(bf16)

    U, V = _bias_factors(np.asarray(dt_bank, f), np.asarray(dh_bank, f),
                         np.asarray(dw_bank, f))
    uconst = np.ascontiguousarray(U.T).astype(bf16)                   # [KB, THW]
    vconst = np.ascontiguousarray(V.transpose(0, 2, 1)).astype(bf16)  # [NH,KB,THW]

    selm = np.zeros((128, KC, 128), np.float32)
    for mf in range(KC):
        for j in range(128):
            h = 2 * mf + j // 64
            selm[32 * (h // 4) + h % 4, mf, j] = 1.0
    selm = selm.astype(bf16)

    flags = (bool(bq.any()), bool(bk.any()), bool(bv.any()),
             bool(b1p.any()), bool(b2p.any()))

    shared = {"wq": wq_e, "wk": wk_e, "wv": wv_e, "wp": wp_e, "w1": w1_e,
              "w2": w2_e, "uconst": uconst, "vconst": vconst, "sel": selm,
              "brows": brows, "bvrow": bvrow}
    in_maps = []
    for i in range(NCORES):
        m = dict(shared)
        m["xs"] = np.ascontiguousarray(xb[i * NBLK:(i + 1) * NBLK])
        in_maps.append(m)
    return flags, in_maps


def gather(results):
    outs = np.concatenate([results[i]["out"][None] for i in range(NCORES)])
    # (NCORES, NBLK, 128, KC, TOK) -> (NB, C, THW) -> (B, C, T, H, W)
    ob = outs.reshape(NB, 128, KC, TOK).transpose(0, 2, 1, 3)
    ob = np.ascontiguousarray(ob).reshape(NB, C, THW)
    ob = ob.reshape(B, ST, SH, SW, C, BT, BH, BW)
    ob = ob.transpose(0, 4, 1, 5, 2, 6, 3, 7).reshape(B, C, T, H, W)
    return np.ascontiguousarray(ob)


def kernel(**inputs):
    from concourse.bass_utils import run_bass_kernel_spmd

    flags, in_maps = prepare(**inputs)
    nc = _get_nc(flags)
    res = run_bass_kernel_spmd(nc, in_maps, list(range(NCORES)))
    return gather(res.results)
